# revision 20
# baseline (speedup 1.0000x reference)
"""Viterbi CRF decode on Trainium2 (Bass), 8-core data-parallel.

Problem: B=128, S=512, T=32 (30 labels + START=30, END=31).
  forward max-plus scan over S steps, backpointers, masked lengths,
  backward pointer-following pass. Output [B, S] int32 tag path.

Sharding: pure data parallel, 16 examples per core.

Per-core layout (SBUF partitions p = 32*q + j, quadrant q in [0,4) holds
examples b = 4q+br, br in [0,4); j in [0,32) is the tag index).

v2: hardware-looped. The per-step work runs inside Fori loops (the
dominant cost on this stack is per-static-instruction, not per-element),
with 8-step unrolled windows so every short-distance dependency uses a
static ring slot; dynamic (register-offset) APs are used only for
write-only archiving (PH/BPW/DEC) and for reads of long-settled data
(FT, XS).

Forward window (8 steps t = 8m..8m+7):
  V2R[s,b,i] = feats[b,t,j(p)] + trans[i,j(p)]   (one op, 1024 el/lane)
  per step: half-split (br 0,1 | 2,3) add + max-reduce + transpose-bcast
  of the new partition vector, with the same op distances the v1 kernel
  proved safe on HW.
  bp extract batched: eq/mul/max-reduce over the whole window (3 ops).
  part_{-1} = one-hot(-1e30) START vector makes step 0 a normal step, so
  there is no special-cased prologue; window 0's bp row k=-1 lands in a
  4-col pad of BPW.

Backward: mailbox ping-pong (M_A/M_B) pointer chase, 4-step unrolled,
one stt + one transpose-prefetch + one archive copy per step.

All compute on the vector engine (exact fp32, same association order as
the jax reference: (feats + trans) + part), DMA on sync engine.
"""

import numpy as np
from contextlib import ExitStack

import concourse.bass as bass
import concourse.mybir as mybir
from concourse.bass import ds
from concourse.bass_utils import run_bass_kernel_spmd

F32 = mybir.dt.float32
I32 = mybir.dt.int32
AX = mybir.AxisListType
OP = mybir.AluOpType

T = 32
START = 30
END = 31
NCORES = 8


def build_nc(S, reps=1):
    nc = bass.Bass(detect_race_conditions=False)
    ft_d = nc.declare_dram_parameter("ft", [128, 4 * S], F32, isOutput=False)
    mkf_d = nc.declare_dram_parameter("mkf", [128, 4 * S + 4], F32, isOutput=False)
    tt_d = nc.declare_dram_parameter("tt", [128, 32], F32, isOutput=False)
    cst_d = nc.declare_dram_parameter("cst", [128, 96], F32, isOutput=False)
    dec_d = nc.declare_dram_parameter("dec", [128, S], I32, isOutput=True)

    K = S - 1           # bp rows k in [0, K)
    W = S // 16         # forward windows (16 steps each)

    with ExitStack() as ctx:
        def sb(name, shape, dt=F32):
            return ctx.enter_context(nc.sbuf_tensor(name, shape, dt))

        FT = sb("FT", [128, 4 * S])
        MKF = sb("MKF", [128, 4 * S + 4])
        TT = sb("TT", [128, 32])
        CST = sb("CST", [128, 96])
        PH = sb("PH", [128, 4 * S])
        PHSR = sb("PHSR", [128, 64])
        V2R = sb("V2R", [128, 2048])
        VR = sb("VR", [128, 2048])
        EB = sb("EB", [128, 2048])
        WB = sb("WB", [128, 2048])
        P4A = sb("P4A", [128, 64])
        P4B = sb("P4B", [128, 64])
        BPW = sb("BPW", [128, 4 * S + 4])   # 4-col pad (bp row k=-1) + 4*K used
        XS = sb("XS", [128, 4 * S + 32])    # +32: backward transposes over-read
        XS2 = sb("XS2", [128, 4 * S])
        ALF = sb("ALF", [128, 4 * S])
        ALB = sb("ALB", [128, 4 * S])
        # DEC strided by 4 so backward-loop APs share one base register
        DEC = sb("DEC", [128, 4 * S])
        DECI = sb("DECI", [128, S], I32)
        TEND = sb("TEND", [128, 32])
        LPP = sb("LPP", [128, 32])
        TLP = sb("TLP", [128, 32])
        CAND = sb("CAND", [128, 32])
        MX = sb("MX", [128, 1])
        EQC = sb("EQC", [128, 32])
        SC = sb("SC", [128, 32])
        PW = sb("PW", [128, 1])
        P32 = sb("P32", [128, 32])
        T32 = sb("T32", [128, 32])
        PR = sb("PR", [128, 32])
        M4 = sb("M4", [128, 4])
        TB = [sb(f"TB{i}", [128, 32]) for i in range(4)]

        with (
            nc.semaphore() as dma_sem,
            nc.semaphore() as done_sem,
            nc.Block() as block,
        ):
            @block.sync
            def _(sync):
                sync.dma_start(out=FT[:], in_=ft_d[:]).then_inc(dma_sem, 16)
                sync.dma_start(out=MKF[:], in_=mkf_d[:]).then_inc(dma_sem, 16)
                sync.dma_start(out=TT[:], in_=tt_d[:]).then_inc(dma_sem, 16)
                sync.dma_start(out=CST[:], in_=cst_d[:]).then_inc(dma_sem, 16)
                sync.wait_ge(done_sem, 1)
                sync.dma_start(out=dec_d[:], in_=DECI[:]).then_inc(dma_sem, 16)

            def emit_body(v):
                # ---- constants / init ----
                v.stream_shuffle(out=TEND[:], in_=TT[:], mask=[END] * 32)
                v.memset(P32[:], 0.0)
                v.memset(LPP[:], 0.0)
                v.memset(XS[:, 4 * S:], 0.0)
                v.tensor_sub(out=ALF[:], in0=MKF[:, 0:4 * S], in1=MKF[:, 4:4 * S + 4])
                v.tensor_scalar(out=ALB[:], in0=ALF[:], scalar1=1.0,
                                scalar2=1e30, op0=OP.subtract, op1=OP.mult)
                # part_{-1}: 0 at START, -1e30 elsewhere -> step 0 is a normal step
                pm1 = CST[:, 64:96].unsqueeze(1).broadcast_to([128, 2, 32])
                v.tensor_copy(out=P4A[:].rearrange("p (b i) -> p b i", b=2), in_=pm1)
                v.tensor_copy(out=P4B[:].rearrange("p (b i) -> p b i", b=2), in_=pm1)

                # ---- forward: flat loop over 8-step windows ----
                v2r_v = V2R[:].rearrange("p (s b i) -> p s b i", b=4, i=32)
                vr_v = VR[:].rearrange("p (s b i) -> p s b i", b=4, i=32)
                eb_v = EB[:].rearrange("p (s b i) -> p s b i", b=4, i=32)
                wb_v = WB[:].rearrange("p (s b i) -> p s b i", b=4, i=32)
                tt_w = TT[:].unsqueeze(1).unsqueeze(1).broadcast_to([128, 16, 4, 32])
                iotad_w = CST[:, 32:64].unsqueeze(1).unsqueeze(1).broadcast_to([128, 16, 4, 32])

                with v.Fori(0, W) as m:
                    mb = m * 64   # single shared dynamic subtree
                    ft_w = FT[:, ds(mb, 64)].rearrange(
                        "p (s b) -> p s b", b=4).unsqueeze(3).broadcast_to([128, 16, 4, 32])
                    v.tensor_tensor(out=v2r_v, in0=ft_w, in1=tt_w, op=OP.add)
                    for j in range(16):
                        # order: [wB, wA, redB, redA, drain, p4B, p4A] keeps
                        # >=2 ops (or a drain) between each write and its
                        # dependent stream read, matching the v1 discipline.
                        for h, P4H in ((1, P4B), (0, P4A)):
                            v.tensor_tensor(
                                out=VR[:, 128 * j + 64 * h:128 * j + 64 * h + 64]
                                    .rearrange("p (b i) -> p b i", b=2),
                                in0=V2R[:, 128 * j + 64 * h:128 * j + 64 * h + 64]
                                    .rearrange("p (b i) -> p b i", b=2),
                                in1=P4H[:].rearrange("p (b i) -> p b i", b=2),
                                op=OP.add)
                        for h, P4H in ((1, P4B), (0, P4A)):
                            v.tensor_reduce(
                                out=PHSR[:, 4 * j + 2 * h:4 * j + 2 * h + 2],
                                in_=VR[:, 128 * j + 64 * h:128 * j + 64 * h + 64]
                                    .rearrange("p (b i) -> p b i", b=2),
                                axis=AX.X, op=OP.max)
                        v.drain()
                        for h, P4H in ((1, P4B), (0, P4A)):
                            v.transpose(
                                out=P4H[:].rearrange("p (b i) -> p b i", b=2),
                                in_=PHSR[:, 4 * j + 2 * h:4 * j + 2 * h + 2]
                                    .unsqueeze(2).broadcast_to([128, 2, 32]))
                    # batched bp extract for the window (rows k = 8m-1 .. 8m+6)
                    phs_w = PHSR[:, 0:64].rearrange(
                        "p (s b) -> p s b", b=4).unsqueeze(3).broadcast_to([128, 16, 4, 32])
                    v.tensor_tensor(out=eb_v, in0=vr_v, in1=phs_w, op=OP.is_equal)
                    v.tensor_tensor(out=wb_v, in0=eb_v, in1=iotad_w, op=OP.mult)
                    v.tensor_reduce(out=BPW[:, ds(mb, 64)], in_=wb_v,
                                    axis=AX.X, op=OP.max)
                    v.tensor_copy(out=PH[:, ds(mb, 64)], in_=PHSR[:, 0:64])

                # ---- last partition: max over t of PH + ALB (by-i-partition) ----
                ph_bt = PH[:, 0:4 * S].rearrange("p (t b) -> p b t", b=4)
                alb_bt = ALB[:].rearrange("p (t b) -> p b t", b=4)
                xs_bt = XS[:, 0:4 * S].rearrange("p (t b) -> p b t", b=4)
                v.tensor_tensor(out=xs_bt, in0=ph_bt, in1=alb_bt, op=OP.add)
                v.tensor_reduce(out=LPP[:, 0:4], in_=xs_bt, axis=AX.X, op=OP.max)

                # bp decode + mask (also serves as filler before TLP reads LPP)
                v.tensor_scalar(out=XS2[:, 0:4 * K], in0=BPW[:, 4:4 * K + 4],
                                scalar1=-1.0, scalar2=31.0, op0=OP.mult, op1=OP.add)
                v.tensor_tensor(out=BPW[:, 4:4 * K + 4], in0=XS2[:, 0:4 * K],
                                in1=MKF[:, 4:4 * K + 4], op=OP.mult)

                # pointer = argmax_i(LP[b,i] + trans[i,END])
                v.transpose(out=TLP[:], in_=LPP[:])
                v.drain()
                v.tensor_tensor(out=CAND[:], in0=TLP[:], in1=TEND[:], op=OP.add)
                v.tensor_reduce(out=MX[:], in_=CAND[:], axis=AX.X, op=OP.max)
                v.drain()
                v.tensor_tensor(out=EQC[:], in0=CAND[:],
                                in1=MX[:].broadcast_to([128, 32]), op=OP.is_equal)
                v.tensor_tensor(out=SC[:], in0=EQC[:], in1=CST[:, 32:64], op=OP.mult)
                v.tensor_reduce(out=PW[:], in_=SC[:], axis=AX.X, op=OP.max)
                v.drain()
                v.tensor_scalar(out=P32[:, 0:1], in0=PW[:], scalar1=-1.0,
                                scalar2=31.0, op0=OP.mult, op1=OP.add)
                v.drain()

                # scatter pointer at k == last_pos: bp' = bp + atlast*(ptr - bp)
                v.transpose(out=T32[:], in_=P32[:])
                v.stream_shuffle(out=PR[:], in_=T32[:], mask=[0] * 32)
                v.drain()
                pr_b = PR[:, 0:4].unsqueeze(1).broadcast_to([128, K, 4])
                bp_v = BPW[:, 4:4 * K + 4].rearrange("p (k b) -> p k b", b=4)
                xs_v = XS[:, 0:4 * K].rearrange("p (k b) -> p k b", b=4)
                xs2_v = XS2[:, 0:4 * K].rearrange("p (k b) -> p k b", b=4)
                alf_v = ALF[:, 0:4 * K].rearrange("p (k b) -> p k b", b=4)
                v.tensor_tensor(out=xs_v, in0=pr_b, in1=bp_v, op=OP.subtract)
                v.tensor_tensor(out=xs2_v, in0=xs_v, in1=alf_v, op=OP.mult)
                v.tensor_tensor(out=xs_v, in0=bp_v, in1=xs2_v, op=OP.add)
                v.drain()

                # ---- backward chase: 4-slot mailbox, 4-step unroll ----
                # DEC[4k] = decode[k]; DEC[4(S-1)] = pointer.
                # Position j writes mailbox col 3-j and reads col (4-j)%4,
                # so one strided copy archives the whole iteration.
                v.tensor_copy(out=DEC[:, 4 * (S - 1):4 * (S - 1) + 1], in_=P32[:, 0:1])
                v.tensor_copy(out=M4[:, 0:1], in_=P32[:, 0:1])
                # prefetch transposes for k = S-2 (slot 2), S-3 (slot 1)
                v.transpose(out=TB[(S - 2) % 4][:], in_=XS[:, 4 * (S - 2):4 * (S - 2) + 32])
                v.transpose(out=TB[(S - 3) % 4][:], in_=XS[:, 4 * (S - 3):4 * (S - 3) + 32])
                # loop covers k = S-2 down to 3; positions j=0..3 handle k-j
                n_it = (K - 3) // 4   # 127 iterations for S=512
                with v.Fori(0, n_it) as mm:
                    base = mm * (-16)   # single shared dynamic subtree
                    for j in range(4):
                        kj = (S - 2 - j) % 4
                        ic = (4 - j) % 4
                        oc = 3 - j
                        v.scalar_tensor_tensor(out=EQC[:], in0=CST[:, 0:32],
                                               scalar=M4[:, ic:ic + 1], in1=TB[kj][:],
                                               op0=OP.is_equal, op1=OP.mult,
                                               accum_out=M4[:, oc:oc + 1])
                        v.transpose(out=TB[(kj - 2) % 4][:],
                                    in_=XS[:, ds(base + (4 * (S - 2 - j) - 8), 32)])
                    v.tensor_copy(
                        out=DEC[:, ds(base + 4 * (S - 5), 16)]
                            .rearrange("p (k f) -> p k f", f=4)[:, :, 0:1],
                        in_=M4[:, 0:4].unsqueeze(2))
                # leftover k = 2, 1, 0: the main loop prefetched steps 2 and 1
                # but slot 0 last held step 4 -- refetch step 0's row.
                # Chain head after the loop sits in M4 col 0 (pos 3's output).
                for n, k in enumerate((2, 1, 0)):
                    ic = (0, 3, 2)[n]
                    oc = (3, 2, 1)[n]
                    v.scalar_tensor_tensor(out=EQC[:], in0=CST[:, 0:32],
                                           scalar=M4[:, ic:ic + 1], in1=TB[k % 4][:],
                                           op0=OP.is_equal, op1=OP.mult,
                                           accum_out=M4[:, oc:oc + 1])
                    if k == 2:
                        v.transpose(out=TB[0][:], in_=XS[:, 0:32])
                    else:
                        v.drain()
                    v.tensor_copy(out=DEC[:, 4 * k:4 * k + 1], in_=M4[:, oc:oc + 1])

                v.drain()
                v.tensor_copy(out=DECI[:].unsqueeze(2),
                              in_=DEC[:].rearrange("p (k f) -> p k f", f=4)[:, :, 0:1])

            @block.vector
            def _(v):
                v.wait_ge(dma_sem, 64)
                # reps via an outer hardware loop: the reps=1 and reps=R
                # programs are byte-identical except the trip count, so
                # wall-clock differencing isolates pure body execution time
                # (and the register cost stays at one body's worth).
                with v.Fori(0, reps) as _r:
                    emit_body(v)
                v.drain().then_inc(done_sem, 1)

    return nc


def pack_inputs(feats, transitions, mask, S):
    """Host-side layout packing (pure data movement, no arithmetic beyond
    dtype conversion of the 0/1 mask)."""
    trans = np.ascontiguousarray(np.asarray(transitions, np.float32))
    ttrep = np.ascontiguousarray(np.tile(trans.T, (4, 1)))  # [128, 32]
    iota = np.arange(32, dtype=np.float32)
    pm1 = np.full(32, -1e30, dtype=np.float32)
    pm1[START] = 0.0
    cst = np.ascontiguousarray(
        np.tile(np.concatenate([iota, 31.0 - iota, pm1])[None, :], (128, 1)))
    in_maps = []
    bc = 16
    for c in range(NCORES):
        f = np.asarray(feats[bc * c:bc * c + bc], np.float32)  # [16, S, 32]
        ft = np.ascontiguousarray(
            f.reshape(4, 4, S, T).transpose(0, 3, 2, 1).reshape(128, 4 * S))
        m = np.asarray(mask[bc * c:bc * c + bc]).astype(np.float32)  # [16, S]
        mk = np.broadcast_to(
            m.reshape(4, 1, 4, S).transpose(0, 1, 3, 2), (4, 32, S, 4))
        mk = mk.reshape(128, 4 * S)
        mkp = np.zeros((128, 4 * S + 4), np.float32)
        mkp[:, :4 * S] = mk
        in_maps.append({"ft": ft, "mkf": mkp, "tt": ttrep, "cst": cst})
    return in_maps


def unpack_outputs(results, S):
    out = np.empty((128, S), np.int32)
    bc = 16
    for c in range(NCORES):
        d = np.asarray(results[c]["dec"]).reshape(4, 32, S)
        out[bc * c:bc * c + bc] = d[:, 0:4, :].reshape(16, S)
    return out


_NC_CACHE = {}


def kernel(feats, transitions, mask):
    B, S, Tin = feats.shape
    assert (B, Tin) == (128, 32)
    if S not in _NC_CACHE:
        _NC_CACHE[S] = build_nc(S)
    nc = _NC_CACHE[S]
    in_maps = pack_inputs(feats, transitions, mask, S)
    res = run_bass_kernel_spmd(nc, in_maps, list(range(NCORES)))
    return unpack_outputs(res.results, S)


# revision 21
# speedup vs baseline: 1.8509x; 1.8509x over previous
"""Viterbi CRF decode on Trainium2 (Bass), 8-core data-parallel.

Problem: B=128, S=512, T=32 (30 labels + START=30, END=31).
  forward max-plus scan over S steps, backpointers, masked lengths,
  backward pointer-following pass. Output [B, S] int32 tag path.

Sharding: pure data parallel, 16 examples per core.

Per-core layout (SBUF partitions p = 32*q + j, quadrant q in [0,4) holds
examples b = 4q+br, br in [0,4); j in [0,32) is the tag index).

v2: hardware-looped. The per-step work runs inside Fori loops (the
dominant cost on this stack is per-static-instruction, not per-element),
with 8-step unrolled windows so every short-distance dependency uses a
static ring slot; dynamic (register-offset) APs are used only for
write-only archiving (PH/BPW/DEC) and for reads of long-settled data
(FT, XS).

Forward window (8 steps t = 8m..8m+7):
  V2R[s,b,i] = feats[b,t,j(p)] + trans[i,j(p)]   (one op, 1024 el/lane)
  per step: half-split (br 0,1 | 2,3) add + max-reduce + transpose-bcast
  of the new partition vector, with the same op distances the v1 kernel
  proved safe on HW.
  bp extract batched: eq/mul/max-reduce over the whole window (3 ops).
  part_{-1} = one-hot(-1e30) START vector makes step 0 a normal step, so
  there is no special-cased prologue; window 0's bp row k=-1 lands in a
  4-col pad of BPW.

Backward: mailbox ping-pong (M_A/M_B) pointer chase, 4-step unrolled,
one stt + one transpose-prefetch + one archive copy per step.

All compute on the vector engine (exact fp32, same association order as
the jax reference: (feats + trans) + part), DMA on sync engine.
"""

import numpy as np
from contextlib import ExitStack

import concourse.bass as bass
import concourse.mybir as mybir
from concourse.bass import ds
from concourse.bass_utils import run_bass_kernel_spmd

F32 = mybir.dt.float32
I32 = mybir.dt.int32
AX = mybir.AxisListType
OP = mybir.AluOpType

T = 32
START = 30
END = 31
NCORES = 8


def build_nc(S, reps=1):
    nc = bass.Bass(detect_race_conditions=False)
    ft_d = nc.declare_dram_parameter("ft", [128, 4 * S], F32, isOutput=False)
    mkf_d = nc.declare_dram_parameter("mkf", [128, 4 * S + 4], F32, isOutput=False)
    tt_d = nc.declare_dram_parameter("tt", [128, 32], F32, isOutput=False)
    cst_d = nc.declare_dram_parameter("cst", [128, 96], F32, isOutput=False)
    dec_d = nc.declare_dram_parameter("dec", [128, S], I32, isOutput=True)

    K = S - 1           # bp rows k in [0, K)
    W = S // 8          # forward windows

    with ExitStack() as ctx:
        def sb(name, shape, dt=F32):
            return ctx.enter_context(nc.sbuf_tensor(name, shape, dt))

        FT = sb("FT", [128, 4 * S])
        MKF = sb("MKF", [128, 4 * S + 4])
        TT = sb("TT", [128, 32])
        CST = sb("CST", [128, 96])
        PH = sb("PH", [128, 4 * S])
        PHSR = sb("PHSR", [128, 32])
        V2R = sb("V2R", [128, 1024])
        VR = sb("VR", [128, 1024])
        EB = sb("EB", [128, 1024])
        WB = sb("WB", [128, 1024])
        P4A = sb("P4A", [128, 64])
        P4B = sb("P4B", [128, 64])
        BPW = sb("BPW", [128, 4 * S + 4])   # 4-col pad (bp row k=-1) + 4*K used
        XS = sb("XS", [128, 4 * S + 32])    # +32: backward transposes over-read
        XS2 = sb("XS2", [128, 4 * S])
        ALF = sb("ALF", [128, 4 * S])
        ALB = sb("ALB", [128, 4 * S])
        # DEC strided by 4 so backward-loop APs share one base register
        DEC = sb("DEC", [128, 4 * S])
        DECI = sb("DECI", [128, S], I32)
        TEND = sb("TEND", [128, 32])
        LPP = sb("LPP", [128, 32])
        TLP = sb("TLP", [128, 32])
        CAND = sb("CAND", [128, 32])
        MX = sb("MX", [128, 1])
        EQC = sb("EQC", [128, 32])
        SC = sb("SC", [128, 32])
        PW = sb("PW", [128, 1])
        P32 = sb("P32", [128, 32])
        T32 = sb("T32", [128, 32])
        PR = sb("PR", [128, 32])
        M4 = sb("M4", [128, 4])
        TB = [sb(f"TB{i}", [128, 32]) for i in range(4)]

        with (
            nc.semaphore() as dma_sem,
            nc.semaphore() as done_sem,
            nc.Block() as block,
        ):
            @block.sync
            def _(sync):
                sync.dma_start(out=FT[:], in_=ft_d[:]).then_inc(dma_sem, 16)
                sync.dma_start(out=MKF[:], in_=mkf_d[:]).then_inc(dma_sem, 16)
                sync.dma_start(out=TT[:], in_=tt_d[:]).then_inc(dma_sem, 16)
                sync.dma_start(out=CST[:], in_=cst_d[:]).then_inc(dma_sem, 16)
                sync.wait_ge(done_sem, 1)
                sync.dma_start(out=dec_d[:], in_=DECI[:]).then_inc(dma_sem, 16)

            def emit_body(v):
                # ---- constants / init ----
                v.stream_shuffle(out=TEND[:], in_=TT[:], mask=[END] * 32)
                v.memset(P32[:], 0.0)
                v.memset(LPP[:], 0.0)
                v.memset(XS[:, 4 * S:], 0.0)
                v.tensor_sub(out=ALF[:], in0=MKF[:, 0:4 * S], in1=MKF[:, 4:4 * S + 4])
                v.tensor_scalar(out=ALB[:], in0=ALF[:], scalar1=1.0,
                                scalar2=1e30, op0=OP.subtract, op1=OP.mult)
                # part_{-1}: 0 at START, -1e30 elsewhere -> step 0 is a normal step
                pm1 = CST[:, 64:96].unsqueeze(1).broadcast_to([128, 2, 32])
                v.tensor_copy(out=P4A[:].rearrange("p (b i) -> p b i", b=2), in_=pm1)
                v.tensor_copy(out=P4B[:].rearrange("p (b i) -> p b i", b=2), in_=pm1)

                # ---- forward: flat loop over 8-step windows ----
                v2r_v = V2R[:].rearrange("p (s b i) -> p s b i", b=4, i=32)
                vr_v = VR[:].rearrange("p (s b i) -> p s b i", b=4, i=32)
                eb_v = EB[:].rearrange("p (s b i) -> p s b i", b=4, i=32)
                wb_v = WB[:].rearrange("p (s b i) -> p s b i", b=4, i=32)
                tt_w = TT[:].unsqueeze(1).unsqueeze(1).broadcast_to([128, 8, 4, 32])
                iotad_w = CST[:, 32:64].unsqueeze(1).unsqueeze(1).broadcast_to([128, 8, 4, 32])

                with v.Fori(0, W) as m:
                    mb = m * 32   # single shared dynamic subtree
                    ft_w = FT[:, ds(mb, 32)].rearrange(
                        "p (s b) -> p s b", b=4).unsqueeze(3).broadcast_to([128, 8, 4, 32])
                    v.tensor_tensor(out=v2r_v, in0=ft_w, in1=tt_w, op=OP.add)
                    for j in range(8):
                        # order: [wB, wA, redB, redA, drain, p4B, p4A] keeps
                        # >=2 ops (or a drain) between each write and its
                        # dependent stream read, matching the v1 discipline.
                        for h, P4H in ((1, P4B), (0, P4A)):
                            v.tensor_tensor(
                                out=VR[:, 128 * j + 64 * h:128 * j + 64 * h + 64]
                                    .rearrange("p (b i) -> p b i", b=2),
                                in0=V2R[:, 128 * j + 64 * h:128 * j + 64 * h + 64]
                                    .rearrange("p (b i) -> p b i", b=2),
                                in1=P4H[:].rearrange("p (b i) -> p b i", b=2),
                                op=OP.add)
                        for h, P4H in ((1, P4B), (0, P4A)):
                            v.tensor_reduce(
                                out=PHSR[:, 4 * j + 2 * h:4 * j + 2 * h + 2],
                                in_=VR[:, 128 * j + 64 * h:128 * j + 64 * h + 64]
                                    .rearrange("p (b i) -> p b i", b=2),
                                axis=AX.X, op=OP.max)
                        v.drain()
                        for h, P4H in ((1, P4B), (0, P4A)):
                            v.transpose(
                                out=P4H[:].rearrange("p (b i) -> p b i", b=2),
                                in_=PHSR[:, 4 * j + 2 * h:4 * j + 2 * h + 2]
                                    .unsqueeze(2).broadcast_to([128, 2, 32]))
                    # batched bp extract for the window (rows k = 8m-1 .. 8m+6)
                    phs_w = PHSR[:, 0:32].rearrange(
                        "p (s b) -> p s b", b=4).unsqueeze(3).broadcast_to([128, 8, 4, 32])
                    v.tensor_tensor(out=eb_v, in0=vr_v, in1=phs_w, op=OP.is_equal)
                    v.tensor_tensor(out=wb_v, in0=eb_v, in1=iotad_w, op=OP.mult)
                    v.tensor_reduce(out=BPW[:, ds(mb, 32)], in_=wb_v,
                                    axis=AX.X, op=OP.max)
                    v.tensor_copy(out=PH[:, ds(mb, 32)], in_=PHSR[:, 0:32])

                # ---- last partition: max over t of PH + ALB (by-i-partition) ----
                ph_bt = PH[:, 0:4 * S].rearrange("p (t b) -> p b t", b=4)
                alb_bt = ALB[:].rearrange("p (t b) -> p b t", b=4)
                xs_bt = XS[:, 0:4 * S].rearrange("p (t b) -> p b t", b=4)
                v.tensor_tensor(out=xs_bt, in0=ph_bt, in1=alb_bt, op=OP.add)
                v.tensor_reduce(out=LPP[:, 0:4], in_=xs_bt, axis=AX.X, op=OP.max)

                # bp decode + mask (also serves as filler before TLP reads LPP)
                v.tensor_scalar(out=XS2[:, 0:4 * K], in0=BPW[:, 4:4 * K + 4],
                                scalar1=-1.0, scalar2=31.0, op0=OP.mult, op1=OP.add)
                v.tensor_tensor(out=BPW[:, 4:4 * K + 4], in0=XS2[:, 0:4 * K],
                                in1=MKF[:, 4:4 * K + 4], op=OP.mult)

                # pointer = argmax_i(LP[b,i] + trans[i,END])
                v.transpose(out=TLP[:], in_=LPP[:])
                v.drain()
                v.tensor_tensor(out=CAND[:], in0=TLP[:], in1=TEND[:], op=OP.add)
                v.tensor_reduce(out=MX[:], in_=CAND[:], axis=AX.X, op=OP.max)
                v.drain()
                v.tensor_tensor(out=EQC[:], in0=CAND[:],
                                in1=MX[:].broadcast_to([128, 32]), op=OP.is_equal)
                v.tensor_tensor(out=SC[:], in0=EQC[:], in1=CST[:, 32:64], op=OP.mult)
                v.tensor_reduce(out=PW[:], in_=SC[:], axis=AX.X, op=OP.max)
                v.drain()
                v.tensor_scalar(out=P32[:, 0:1], in0=PW[:], scalar1=-1.0,
                                scalar2=31.0, op0=OP.mult, op1=OP.add)
                v.drain()

                # scatter pointer at k == last_pos: bp' = bp + atlast*(ptr - bp)
                v.transpose(out=T32[:], in_=P32[:])
                v.stream_shuffle(out=PR[:], in_=T32[:], mask=[0] * 32)
                v.drain()
                pr_b = PR[:, 0:4].unsqueeze(1).broadcast_to([128, K, 4])
                bp_v = BPW[:, 4:4 * K + 4].rearrange("p (k b) -> p k b", b=4)
                xs_v = XS[:, 0:4 * K].rearrange("p (k b) -> p k b", b=4)
                xs2_v = XS2[:, 0:4 * K].rearrange("p (k b) -> p k b", b=4)
                alf_v = ALF[:, 0:4 * K].rearrange("p (k b) -> p k b", b=4)
                v.tensor_tensor(out=xs_v, in0=pr_b, in1=bp_v, op=OP.subtract)
                v.tensor_tensor(out=xs2_v, in0=xs_v, in1=alf_v, op=OP.mult)
                v.tensor_tensor(out=xs_v, in0=bp_v, in1=xs2_v, op=OP.add)
                v.drain()

                # ---- backward chase: 4-slot mailbox, 4-step unroll ----
                # DEC[4k] = decode[k]; DEC[4(S-1)] = pointer.
                # Position j writes mailbox col 3-j and reads col (4-j)%4,
                # so one strided copy archives the whole iteration.
                v.tensor_copy(out=DEC[:, 4 * (S - 1):4 * (S - 1) + 1], in_=P32[:, 0:1])
                v.tensor_copy(out=M4[:, 0:1], in_=P32[:, 0:1])
                # prefetch transposes for k = S-2 (slot 2), S-3 (slot 1)
                v.transpose(out=TB[(S - 2) % 4][:], in_=XS[:, 4 * (S - 2):4 * (S - 2) + 32])
                v.transpose(out=TB[(S - 3) % 4][:], in_=XS[:, 4 * (S - 3):4 * (S - 3) + 32])
                # loop covers k = S-2 down to 3; positions j=0..3 handle k-j
                n_it = (K - 3) // 4   # 127 iterations for S=512
                with v.Fori(0, n_it) as mm:
                    base = mm * (-16)   # single shared dynamic subtree
                    for j in range(4):
                        kj = (S - 2 - j) % 4
                        ic = (4 - j) % 4
                        oc = 3 - j
                        v.scalar_tensor_tensor(out=EQC[:], in0=CST[:, 0:32],
                                               scalar=M4[:, ic:ic + 1], in1=TB[kj][:],
                                               op0=OP.is_equal, op1=OP.mult,
                                               accum_out=M4[:, oc:oc + 1])
                        v.transpose(out=TB[(kj - 2) % 4][:],
                                    in_=XS[:, ds(base + (4 * (S - 2 - j) - 8), 32)])
                    v.tensor_copy(
                        out=DEC[:, ds(base + 4 * (S - 5), 16)]
                            .rearrange("p (k f) -> p k f", f=4)[:, :, 0:1],
                        in_=M4[:, 0:4].unsqueeze(2))
                # leftover k = 2, 1, 0: the main loop prefetched steps 2 and 1
                # but slot 0 last held step 4 -- refetch step 0's row.
                # Chain head after the loop sits in M4 col 0 (pos 3's output).
                for n, k in enumerate((2, 1, 0)):
                    ic = (0, 3, 2)[n]
                    oc = (3, 2, 1)[n]
                    v.scalar_tensor_tensor(out=EQC[:], in0=CST[:, 0:32],
                                           scalar=M4[:, ic:ic + 1], in1=TB[k % 4][:],
                                           op0=OP.is_equal, op1=OP.mult,
                                           accum_out=M4[:, oc:oc + 1])
                    if k == 2:
                        v.transpose(out=TB[0][:], in_=XS[:, 0:32])
                    else:
                        v.drain()
                    v.tensor_copy(out=DEC[:, 4 * k:4 * k + 1], in_=M4[:, oc:oc + 1])

                v.drain()
                v.tensor_copy(out=DECI[:].unsqueeze(2),
                              in_=DEC[:].rearrange("p (k f) -> p k f", f=4)[:, :, 0:1])

            @block.vector
            def _(v):
                v.wait_ge(dma_sem, 64)
                # reps via an outer hardware loop: the reps=1 and reps=R
                # programs are byte-identical except the trip count, so
                # wall-clock differencing isolates pure body execution time
                # (and the register cost stays at one body's worth).
                with v.Fori(0, reps) as _r:
                    emit_body(v)
                v.drain().then_inc(done_sem, 1)

    return nc


def pack_inputs(feats, transitions, mask, S):
    """Host-side layout packing (pure data movement, no arithmetic beyond
    dtype conversion of the 0/1 mask)."""
    trans = np.ascontiguousarray(np.asarray(transitions, np.float32))
    ttrep = np.ascontiguousarray(np.tile(trans.T, (4, 1)))  # [128, 32]
    iota = np.arange(32, dtype=np.float32)
    pm1 = np.full(32, -1e30, dtype=np.float32)
    pm1[START] = 0.0
    cst = np.ascontiguousarray(
        np.tile(np.concatenate([iota, 31.0 - iota, pm1])[None, :], (128, 1)))
    in_maps = []
    bc = 16
    for c in range(NCORES):
        f = np.asarray(feats[bc * c:bc * c + bc], np.float32)  # [16, S, 32]
        ft = np.ascontiguousarray(
            f.reshape(4, 4, S, T).transpose(0, 3, 2, 1).reshape(128, 4 * S))
        m = np.asarray(mask[bc * c:bc * c + bc]).astype(np.float32)  # [16, S]
        mk = np.broadcast_to(
            m.reshape(4, 1, 4, S).transpose(0, 1, 3, 2), (4, 32, S, 4))
        mk = mk.reshape(128, 4 * S)
        mkp = np.zeros((128, 4 * S + 4), np.float32)
        mkp[:, :4 * S] = mk
        in_maps.append({"ft": ft, "mkf": mkp, "tt": ttrep, "cst": cst})
    return in_maps


def unpack_outputs(results, S):
    out = np.empty((128, S), np.int32)
    bc = 16
    for c in range(NCORES):
        d = np.asarray(results[c]["dec"]).reshape(4, 32, S)
        out[bc * c:bc * c + bc] = d[:, 0:4, :].reshape(16, S)
    return out


_NC_CACHE = {}


def kernel(feats, transitions, mask):
    B, S, Tin = feats.shape
    assert (B, Tin) == (128, 32)
    if S not in _NC_CACHE:
        _NC_CACHE[S] = build_nc(S)
    nc = _NC_CACHE[S]
    in_maps = pack_inputs(feats, transitions, mask, S)
    res = run_bass_kernel_spmd(nc, in_maps, list(range(NCORES)))
    return unpack_outputs(res.results, S)


# revision 22
# speedup vs baseline: 1.9155x; 1.0349x over previous
"""Viterbi CRF decode on Trainium2 (Bass), 8-core data-parallel.

Problem: B=128, S=512, T=32 (30 labels + START=30, END=31).
  forward max-plus scan over S steps, backpointers, masked lengths,
  backward pointer-following pass. Output [B, S] int32 tag path.

Sharding: pure data parallel, 16 examples per core.

Per-core layout (SBUF partitions p = 32*q + j, quadrant q in [0,4) holds
examples b = 4q+br, br in [0,4); j in [0,32) is the tag index).

v2: hardware-looped. The per-step work runs inside Fori loops (the
dominant cost on this stack is per-static-instruction, not per-element),
with 8-step unrolled windows so every short-distance dependency uses a
static ring slot; dynamic (register-offset) APs are used only for
write-only archiving (PH/BPW/DEC) and for reads of long-settled data
(FT, XS).

Forward window (8 steps t = 8m..8m+7):
  V2R[s,b,i] = feats[b,t,j(p)] + trans[i,j(p)]   (one op, 1024 el/lane)
  per step: half-split (br 0,1 | 2,3) add + max-reduce + transpose-bcast
  of the new partition vector, with the same op distances the v1 kernel
  proved safe on HW.
  bp extract batched: eq/mul/max-reduce over the whole window (3 ops).
  part_{-1} = one-hot(-1e30) START vector makes step 0 a normal step, so
  there is no special-cased prologue; window 0's bp row k=-1 lands in a
  4-col pad of BPW.

Backward: mailbox ping-pong (M_A/M_B) pointer chase, 4-step unrolled,
one stt + one transpose-prefetch + one archive copy per step.

All compute on the vector engine (exact fp32, same association order as
the jax reference: (feats + trans) + part), DMA on sync engine.
"""

import numpy as np
from contextlib import ExitStack

import concourse.bass as bass
import concourse.mybir as mybir
from concourse.bass import ds
from concourse.bass_utils import run_bass_kernel_spmd

F32 = mybir.dt.float32
I32 = mybir.dt.int32
AX = mybir.AxisListType
OP = mybir.AluOpType

T = 32
START = 30
END = 31
NCORES = 8


def build_nc(S, reps=1):
    nc = bass.Bass(detect_race_conditions=False)
    ft_d = nc.declare_dram_parameter("ft", [128, 4 * S], F32, isOutput=False)
    mkf_d = nc.declare_dram_parameter("mkf", [128, 4 * S + 4], F32, isOutput=False)
    tt_d = nc.declare_dram_parameter("tt", [128, 32], F32, isOutput=False)
    cst_d = nc.declare_dram_parameter("cst", [128, 96], F32, isOutput=False)
    dec_d = nc.declare_dram_parameter("dec", [128, S], I32, isOutput=True)

    K = S - 1           # bp rows k in [0, K)
    W = S // 8          # forward windows

    with ExitStack() as ctx:
        def sb(name, shape, dt=F32):
            return ctx.enter_context(nc.sbuf_tensor(name, shape, dt))

        FT = sb("FT", [128, 4 * S])
        MKF = sb("MKF", [128, 4 * S + 4])
        TT = sb("TT", [128, 32])
        CST = sb("CST", [128, 96])
        PH = sb("PH", [128, 4 * S])
        PHSR = sb("PHSR", [128, 32])
        V2R = sb("V2R", [128, 1024])
        VR = sb("VR", [128, 1024])
        EB = sb("EB", [128, 1024])
        WB = sb("WB", [128, 1024])
        P4A = sb("P4A", [128, 64])
        P4B = sb("P4B", [128, 64])
        BPW = sb("BPW", [128, 4 * S + 4])   # 4-col pad (bp row k=-1) + 4*K used
        XS = sb("XS", [128, 4 * S + 32])    # +32: backward transposes over-read
        XS2 = sb("XS2", [128, 4 * S])
        ALF = sb("ALF", [128, 4 * S])
        ALB = sb("ALB", [128, 4 * S])
        # DEC strided by 4 so backward-loop APs share one base register
        DEC = sb("DEC", [128, 4 * S])
        DECI = sb("DECI", [128, S], I32)
        TEND = sb("TEND", [128, 32])
        LPP = sb("LPP", [128, 32])
        TLP = sb("TLP", [128, 32])
        CAND = sb("CAND", [128, 32])
        MX = sb("MX", [128, 1])
        EQC = sb("EQC", [128, 32])
        SC = sb("SC", [128, 32])
        PW = sb("PW", [128, 1])
        P32 = sb("P32", [128, 32])
        T32 = sb("T32", [128, 32])
        PR = sb("PR", [128, 32])
        M4 = sb("M4", [128, 4])
        TB = [sb(f"TB{i}", [128, 32]) for i in range(4)]

        with (
            nc.semaphore() as dma_sem,
            nc.semaphore() as done_sem,
            nc.Block() as block,
        ):
            @block.sync
            def _(sync):
                sync.dma_start(out=FT[:], in_=ft_d[:]).then_inc(dma_sem, 16)
                sync.dma_start(out=MKF[:], in_=mkf_d[:]).then_inc(dma_sem, 16)
                sync.dma_start(out=TT[:], in_=tt_d[:]).then_inc(dma_sem, 16)
                sync.dma_start(out=CST[:], in_=cst_d[:]).then_inc(dma_sem, 16)
                sync.wait_ge(done_sem, 1)
                sync.dma_start(out=dec_d[:], in_=DECI[:]).then_inc(dma_sem, 16)

            def emit_body(v):
                # ---- constants / init ----
                v.stream_shuffle(out=TEND[:], in_=TT[:], mask=[END] * 32)
                v.memset(P32[:], 0.0)
                v.memset(LPP[:], 0.0)
                v.memset(XS[:, 4 * S:], 0.0)
                v.tensor_sub(out=ALF[:], in0=MKF[:, 0:4 * S], in1=MKF[:, 4:4 * S + 4])
                v.tensor_scalar(out=ALB[:], in0=ALF[:], scalar1=1.0,
                                scalar2=1e30, op0=OP.subtract, op1=OP.mult)
                # part_{-1}: 0 at START, -1e30 elsewhere -> step 0 is a normal step
                pm1 = CST[:, 64:96].unsqueeze(1).broadcast_to([128, 2, 32])
                v.tensor_copy(out=P4A[:].rearrange("p (b i) -> p b i", b=2), in_=pm1)
                v.tensor_copy(out=P4B[:].rearrange("p (b i) -> p b i", b=2), in_=pm1)

                # ---- forward: flat loop over 8-step windows ----
                v2r_v = V2R[:].rearrange("p (s b i) -> p s b i", b=4, i=32)
                vr_v = VR[:].rearrange("p (s b i) -> p s b i", b=4, i=32)
                eb_v = EB[:].rearrange("p (s b i) -> p s b i", b=4, i=32)
                wb_v = WB[:].rearrange("p (s b i) -> p s b i", b=4, i=32)
                tt_w = TT[:].unsqueeze(1).unsqueeze(1).broadcast_to([128, 8, 4, 32])
                iotad_w = CST[:, 32:64].unsqueeze(1).unsqueeze(1).broadcast_to([128, 8, 4, 32])

                with v.Fori(0, W) as m:
                    mb = m * 32   # single shared dynamic subtree
                    ft_w = FT[:, ds(mb, 32)].rearrange(
                        "p (s b) -> p s b", b=4).unsqueeze(3).broadcast_to([128, 8, 4, 32])
                    v.tensor_tensor(out=v2r_v, in0=ft_w, in1=tt_w, op=OP.add)
                    for j in range(8):
                        # order: [wB, wA, redB, redA, drain, p4B, p4A] keeps
                        # >=2 ops (or a drain) between each write and its
                        # dependent stream read, matching the v1 discipline.
                        for h, P4H in ((1, P4B), (0, P4A)):
                            v.tensor_tensor(
                                out=VR[:, 128 * j + 64 * h:128 * j + 64 * h + 64]
                                    .rearrange("p (b i) -> p b i", b=2),
                                in0=V2R[:, 128 * j + 64 * h:128 * j + 64 * h + 64]
                                    .rearrange("p (b i) -> p b i", b=2),
                                in1=P4H[:].rearrange("p (b i) -> p b i", b=2),
                                op=OP.add)
                        v.tensor_reduce(
                            out=PHSR[:, 4 * j:4 * j + 4],
                            in_=VR[:, 128 * j:128 * j + 128]
                                .rearrange("p (b i) -> p b i", b=4),
                            axis=AX.X, op=OP.max)
                        v.drain()
                        for h, P4H in ((1, P4B), (0, P4A)):
                            v.transpose(
                                out=P4H[:].rearrange("p (b i) -> p b i", b=2),
                                in_=PHSR[:, 4 * j + 2 * h:4 * j + 2 * h + 2]
                                    .unsqueeze(2).broadcast_to([128, 2, 32]))
                    # batched bp extract for the window (rows k = 8m-1 .. 8m+6)
                    phs_w = PHSR[:, 0:32].rearrange(
                        "p (s b) -> p s b", b=4).unsqueeze(3).broadcast_to([128, 8, 4, 32])
                    v.tensor_tensor(out=eb_v, in0=vr_v, in1=phs_w, op=OP.is_equal)
                    v.tensor_tensor(out=wb_v, in0=eb_v, in1=iotad_w, op=OP.mult)
                    v.tensor_reduce(out=BPW[:, ds(mb, 32)], in_=wb_v,
                                    axis=AX.X, op=OP.max)
                    v.tensor_copy(out=PH[:, ds(mb, 32)], in_=PHSR[:, 0:32])

                # ---- last partition: max over t of PH + ALB (by-i-partition) ----
                ph_bt = PH[:, 0:4 * S].rearrange("p (t b) -> p b t", b=4)
                alb_bt = ALB[:].rearrange("p (t b) -> p b t", b=4)
                xs_bt = XS[:, 0:4 * S].rearrange("p (t b) -> p b t", b=4)
                v.tensor_tensor(out=xs_bt, in0=ph_bt, in1=alb_bt, op=OP.add)
                v.tensor_reduce(out=LPP[:, 0:4], in_=xs_bt, axis=AX.X, op=OP.max)

                # bp decode + mask (also serves as filler before TLP reads LPP)
                v.tensor_scalar(out=XS2[:, 0:4 * K], in0=BPW[:, 4:4 * K + 4],
                                scalar1=-1.0, scalar2=31.0, op0=OP.mult, op1=OP.add)
                v.tensor_tensor(out=BPW[:, 4:4 * K + 4], in0=XS2[:, 0:4 * K],
                                in1=MKF[:, 4:4 * K + 4], op=OP.mult)

                # pointer = argmax_i(LP[b,i] + trans[i,END])
                v.transpose(out=TLP[:], in_=LPP[:])
                v.drain()
                v.tensor_tensor(out=CAND[:], in0=TLP[:], in1=TEND[:], op=OP.add)
                v.tensor_reduce(out=MX[:], in_=CAND[:], axis=AX.X, op=OP.max)
                v.drain()
                v.tensor_tensor(out=EQC[:], in0=CAND[:],
                                in1=MX[:].broadcast_to([128, 32]), op=OP.is_equal)
                v.tensor_tensor(out=SC[:], in0=EQC[:], in1=CST[:, 32:64], op=OP.mult)
                v.tensor_reduce(out=PW[:], in_=SC[:], axis=AX.X, op=OP.max)
                v.drain()
                v.tensor_scalar(out=P32[:, 0:1], in0=PW[:], scalar1=-1.0,
                                scalar2=31.0, op0=OP.mult, op1=OP.add)
                v.drain()

                # scatter pointer at k == last_pos: bp' = bp + atlast*(ptr - bp)
                v.transpose(out=T32[:], in_=P32[:])
                v.stream_shuffle(out=PR[:], in_=T32[:], mask=[0] * 32)
                v.drain()
                pr_b = PR[:, 0:4].unsqueeze(1).broadcast_to([128, K, 4])
                bp_v = BPW[:, 4:4 * K + 4].rearrange("p (k b) -> p k b", b=4)
                xs_v = XS[:, 0:4 * K].rearrange("p (k b) -> p k b", b=4)
                xs2_v = XS2[:, 0:4 * K].rearrange("p (k b) -> p k b", b=4)
                alf_v = ALF[:, 0:4 * K].rearrange("p (k b) -> p k b", b=4)
                v.tensor_tensor(out=xs_v, in0=pr_b, in1=bp_v, op=OP.subtract)
                v.tensor_tensor(out=xs2_v, in0=xs_v, in1=alf_v, op=OP.mult)
                v.tensor_tensor(out=xs_v, in0=bp_v, in1=xs2_v, op=OP.add)
                v.drain()

                # ---- backward chase: 4-slot mailbox, 4-step unroll ----
                # DEC[4k] = decode[k]; DEC[4(S-1)] = pointer.
                # Position j writes mailbox col 3-j and reads col (4-j)%4,
                # so one strided copy archives the whole iteration.
                v.tensor_copy(out=DEC[:, 4 * (S - 1):4 * (S - 1) + 1], in_=P32[:, 0:1])
                v.tensor_copy(out=M4[:, 0:1], in_=P32[:, 0:1])
                # prefetch transposes for k = S-2 (slot 2), S-3 (slot 1)
                v.transpose(out=TB[(S - 2) % 4][:], in_=XS[:, 4 * (S - 2):4 * (S - 2) + 32])
                v.transpose(out=TB[(S - 3) % 4][:], in_=XS[:, 4 * (S - 3):4 * (S - 3) + 32])
                # loop covers k = S-2 down to 3; positions j=0..3 handle k-j
                n_it = (K - 3) // 4   # 127 iterations for S=512
                with v.Fori(0, n_it) as mm:
                    base = mm * (-16)   # single shared dynamic subtree
                    for j in range(4):
                        kj = (S - 2 - j) % 4
                        ic = (4 - j) % 4
                        oc = 3 - j
                        v.scalar_tensor_tensor(out=EQC[:], in0=CST[:, 0:32],
                                               scalar=M4[:, ic:ic + 1], in1=TB[kj][:],
                                               op0=OP.is_equal, op1=OP.mult,
                                               accum_out=M4[:, oc:oc + 1])
                        v.transpose(out=TB[(kj - 2) % 4][:],
                                    in_=XS[:, ds(base + (4 * (S - 2 - j) - 8), 32)])
                    v.tensor_copy(
                        out=DEC[:, ds(base + 4 * (S - 5), 16)]
                            .rearrange("p (k f) -> p k f", f=4)[:, :, 0:1],
                        in_=M4[:, 0:4].unsqueeze(2))
                # leftover k = 2, 1, 0: the main loop prefetched steps 2 and 1
                # but slot 0 last held step 4 -- refetch step 0's row.
                # Chain head after the loop sits in M4 col 0 (pos 3's output).
                for n, k in enumerate((2, 1, 0)):
                    ic = (0, 3, 2)[n]
                    oc = (3, 2, 1)[n]
                    v.scalar_tensor_tensor(out=EQC[:], in0=CST[:, 0:32],
                                           scalar=M4[:, ic:ic + 1], in1=TB[k % 4][:],
                                           op0=OP.is_equal, op1=OP.mult,
                                           accum_out=M4[:, oc:oc + 1])
                    if k == 2:
                        v.transpose(out=TB[0][:], in_=XS[:, 0:32])
                    else:
                        v.drain()
                    v.tensor_copy(out=DEC[:, 4 * k:4 * k + 1], in_=M4[:, oc:oc + 1])

                v.drain()
                v.tensor_copy(out=DECI[:].unsqueeze(2),
                              in_=DEC[:].rearrange("p (k f) -> p k f", f=4)[:, :, 0:1])

            @block.vector
            def _(v):
                v.wait_ge(dma_sem, 64)
                # reps via an outer hardware loop: the reps=1 and reps=R
                # programs are byte-identical except the trip count, so
                # wall-clock differencing isolates pure body execution time
                # (and the register cost stays at one body's worth).
                with v.Fori(0, reps) as _r:
                    emit_body(v)
                v.drain().then_inc(done_sem, 1)

    return nc


def pack_inputs(feats, transitions, mask, S):
    """Host-side layout packing (pure data movement, no arithmetic beyond
    dtype conversion of the 0/1 mask)."""
    trans = np.ascontiguousarray(np.asarray(transitions, np.float32))
    ttrep = np.ascontiguousarray(np.tile(trans.T, (4, 1)))  # [128, 32]
    iota = np.arange(32, dtype=np.float32)
    pm1 = np.full(32, -1e30, dtype=np.float32)
    pm1[START] = 0.0
    cst = np.ascontiguousarray(
        np.tile(np.concatenate([iota, 31.0 - iota, pm1])[None, :], (128, 1)))
    in_maps = []
    bc = 16
    for c in range(NCORES):
        f = np.asarray(feats[bc * c:bc * c + bc], np.float32)  # [16, S, 32]
        ft = np.ascontiguousarray(
            f.reshape(4, 4, S, T).transpose(0, 3, 2, 1).reshape(128, 4 * S))
        m = np.asarray(mask[bc * c:bc * c + bc]).astype(np.float32)  # [16, S]
        mk = np.broadcast_to(
            m.reshape(4, 1, 4, S).transpose(0, 1, 3, 2), (4, 32, S, 4))
        mk = mk.reshape(128, 4 * S)
        mkp = np.zeros((128, 4 * S + 4), np.float32)
        mkp[:, :4 * S] = mk
        in_maps.append({"ft": ft, "mkf": mkp, "tt": ttrep, "cst": cst})
    return in_maps


def unpack_outputs(results, S):
    out = np.empty((128, S), np.int32)
    bc = 16
    for c in range(NCORES):
        d = np.asarray(results[c]["dec"]).reshape(4, 32, S)
        out[bc * c:bc * c + bc] = d[:, 0:4, :].reshape(16, S)
    return out


_NC_CACHE = {}


def kernel(feats, transitions, mask):
    B, S, Tin = feats.shape
    assert (B, Tin) == (128, 32)
    if S not in _NC_CACHE:
        _NC_CACHE[S] = build_nc(S)
    nc = _NC_CACHE[S]
    in_maps = pack_inputs(feats, transitions, mask, S)
    res = run_bass_kernel_spmd(nc, in_maps, list(range(NCORES)))
    return unpack_outputs(res.results, S)
